# revision 10
# baseline (speedup 1.0000x reference)
"""Edge-MLP (GNN message passing) Trainium2 kernel.

Computes, for each edge e = (s, d):
    logit[e] = W3 @ elu(W2 @ elu(W1 @ [x[s]; x[d]] + b1) + b2) + b3

Strategy:
  - Edge-parallel across 8 NeuronCores; node-feature table replicated (bf16).
  - Per-edge rows gathered from HBM with dma_gather(transpose=True), which
    lands feature-major tiles [128 feat, 3, n_edges] in SBUF -- directly
    consumable as matmul rhs (contraction over the feature partition dim).
  - dma_gather indices are int16, so the 100000-row table is addressed in 4
    chunks of 25000 rows; edges are bucketed by (src_chunk, dst_chunk) on the
    host and each bucket is gathered from the right table chunks.
  - ELU has no HW activation; uses the exact identity
        elu(z) + 1 = relu(z) + min(exp(z), 1)
    and compensates the +1 shift in the next layer's bias (b' = b - W.sum(1)).
"""

import numpy as np
import ml_dtypes

H = 384
NODES = 100000
CHUNK = 25000
NCHUNKS = 4
NCORES = 8
ETILE = 512

BF16 = ml_dtypes.bfloat16

# set by test harness if desired
TRACE = False
LAST_RESULTS = None


def _build_program(nbs, NT, b3v):
    import concourse.bacc as bacc
    import concourse.mybir as mybir
    from concourse.tile import TileContext

    AF = mybir.ActivationFunctionType
    ALU = mybir.AluOpType
    bf16 = mybir.dt.bfloat16
    f32 = mybir.dt.float32
    i16 = mybir.dt.int16

    nc = bacc.Bacc("TRN2")
    xt = nc.dram_tensor("xt", [NODES, H], bf16, kind="ExternalInput")
    sidx = nc.dram_tensor("sidx", [128, NT // 16], i16, kind="ExternalInput")
    didx = nc.dram_tensor("didx", [128, NT // 16], i16, kind="ExternalInput")
    w1a = nc.dram_tensor("w1a", [128, 3 * H], bf16, kind="ExternalInput")
    w1b = nc.dram_tensor("w1b", [128, 3 * H], bf16, kind="ExternalInput")
    w2 = nc.dram_tensor("w2", [128, 3 * H], bf16, kind="ExternalInput")
    w3 = nc.dram_tensor("w3", [128, 3], bf16, kind="ExternalInput")
    b1d = nc.dram_tensor("b1", [128, 3], f32, kind="ExternalInput")
    b2d = nc.dram_tensor("b2", [128, 3], f32, kind="ExternalInput")
    b3d = nc.dram_tensor("b3", [1, 1], f32, kind="ExternalInput")
    outd = nc.dram_tensor("out", [1, NT], f32, kind="ExternalOutput")

    offs = np.concatenate([[0], np.cumsum(nbs)]).astype(int)

    with TileContext(nc) as tc:
        with (
            tc.tile_pool(name="wp", bufs=1) as wp,
            tc.tile_pool(name="gp", bufs=3) as gp,
            tc.tile_pool(name="hp", bufs=2) as hp,
            tc.tile_pool(name="op", bufs=4) as opool,
            tc.tile_pool(name="pp", bufs=4, space="PSUM") as pp,
            tc.tile_pool(name="pp3", bufs=2, space="PSUM") as pp3,
        ):
            def load(dram, tg):
                s = wp.tile(list(dram.shape), dram.dtype, tag=tg)
                nc.sync.dma_start(s[:], dram[:])
                return s

            w1a_s = load(w1a, "w1a")
            w1b_s = load(w1b, "w1b")
            w2_s = load(w2, "w2")
            w3_s = load(w3, "w3")
            b1_s = load(b1d, "b1")
            b2_s = load(b2d, "b2")
            b3_s = load(b3d, "b3")
            sidx_s = load(sidx, "sidx")
            didx_s = load(didx, "didx")

            def elu_block(p, hout, bias_ap, n):
                # hout <- relu(p + bias) + min(exp(p + bias), 1)   (= elu+1)
                et = hp.tile([128, ETILE], bf16, tag="et")
                nc.scalar.activation(et[:, :n], p[:, :n], AF.Exp, bias=bias_ap)
                nc.scalar.activation(hout, p[:, :n], AF.Relu, bias=bias_ap)
                nc.vector.tensor_scalar_min(et[:, :n], et[:, :n], 1.0)
                nc.vector.tensor_tensor(hout, hout, et[:, :n], ALU.add)

            for b in range(16):
                nb = int(nbs[b])
                off = int(offs[b])
                if nb == 0:
                    continue
                cs, cd = divmod(b, NCHUNKS)
                for n0 in range(0, nb, ETILE):
                    n = min(ETILE, nb - n0)
                    # gather this chunk's src/dst rows (feature-major).
                    # num_idxs per call kept <=512: the Q7 ucode wedges the
                    # device for calls around ~1k indices.
                    o16 = (off + n0) // 16
                    sg = gp.tile([128, 3, n], bf16, tag="sg")
                    nc.gpsimd.dma_gather(
                        sg[:], xt[cs * CHUNK:(cs + 1) * CHUNK, :],
                        sidx_s[:, o16:o16 + n // 16], n, n, H,
                        transpose=True,
                    )
                    dg = gp.tile([128, 3, n], bf16, tag="dg")
                    nc.gpsimd.dma_gather(
                        dg[:], xt[cd * CHUNK:(cd + 1) * CHUNK, :],
                        didx_s[:, o16:o16 + n // 16], n, n, H,
                        transpose=True,
                    )
                    h1 = hp.tile([128, 3, ETILE], bf16, tag="h1")
                    h2 = hp.tile([128, 3, ETILE], bf16, tag="h2")
                    # layer 1: p = W1a @ x_src + W1b @ x_dst  (accum in PSUM)
                    for mt in range(3):
                        p = pp.tile([128, ETILE], f32, tag="p")
                        for kc in range(3):
                            nc.tensor.matmul(
                                p[:, :n],
                                w1a_s[:, kc * H + mt * 128: kc * H + mt * 128 + 128],
                                sg[:, kc, :n],
                                start=(kc == 0), stop=False,
                            )
                        for kc in range(3):
                            nc.tensor.matmul(
                                p[:, :n],
                                w1b_s[:, kc * H + mt * 128: kc * H + mt * 128 + 128],
                                dg[:, kc, :n],
                                start=False, stop=(kc == 2),
                            )
                        elu_block(p, h1[:, mt, :n], b1_s[:, mt:mt + 1], n)
                    # layer 2
                    for mt in range(3):
                        p = pp.tile([128, ETILE], f32, tag="p")
                        for kc in range(3):
                            nc.tensor.matmul(
                                p[:, :n],
                                w2_s[:, kc * H + mt * 128: kc * H + mt * 128 + 128],
                                h1[:, kc, :n],
                                start=(kc == 0), stop=(kc == 2),
                            )
                        elu_block(p, h2[:, mt, :n], b2_s[:, mt:mt + 1], n)
                    # layer 3: logits [1, n]
                    p3 = pp3.tile([1, ETILE], f32, tag="p3")
                    for kc in range(3):
                        nc.tensor.matmul(
                            p3[:, :n], w3_s[:, kc:kc + 1], h2[:, kc, :n],
                            start=(kc == 0), stop=(kc == 2),
                        )
                    oc = opool.tile([1, ETILE], f32, tag="oc")
                    nc.scalar.activation(oc[:, :n], p3[:, :n], AF.Identity,
                                         bias=b3_s[:, 0:1])
                    nc.sync.dma_start(outd[:, off + n0:off + n0 + n], oc[:, :n])

    nc.finalize()
    return nc


def _prep(x1, x2, edge_index, W1, b1, W2, b2, W3, b3):
    x = np.concatenate([np.asarray(x1), np.asarray(x2)], axis=0).astype(BF16)
    ei = np.asarray(edge_index).astype(np.int64)
    src, dst = ei[0], ei[1]
    E = src.shape[0]

    bucket = (src // CHUNK) * NCHUNKS + (dst // CHUNK)
    order = np.argsort(bucket, kind="stable")
    counts = np.bincount(bucket, minlength=16)
    starts = np.concatenate([[0], np.cumsum(counts)]).astype(int)

    nbs = []
    core_lists = [[None] * 16 for _ in range(NCORES)]
    for b in range(16):
        eb = order[starts[b]:starts[b + 1]]
        mx = -(-len(eb) // NCORES) if len(eb) else 0
        nb = ((mx + 127) // 128) * 128 if mx else 0
        nbs.append(nb)
        for c in range(NCORES):
            core_lists[c][b] = eb[c::NCORES]
    offs = np.concatenate([[0], np.cumsum(nbs)]).astype(int)
    NT = int(offs[-1])

    src16 = np.zeros((NCORES, NT), np.int16)
    dst16 = np.zeros((NCORES, NT), np.int16)
    eid = np.full((NCORES, NT), -1, np.int64)
    for c in range(NCORES):
        for b in range(16):
            e = core_lists[c][b]
            o = int(offs[b])
            ln = len(e)
            if ln:
                src16[c, o:o + ln] = (src[e] % CHUNK).astype(np.int16)
                dst16[c, o:o + ln] = (dst[e] % CHUNK).astype(np.int16)
                eid[c, o:o + ln] = e

    def wrap(a):
        return np.tile(np.ascontiguousarray(a.reshape(-1, 16).T), (8, 1))

    W1f = np.asarray(W1, np.float64)
    W2f = np.asarray(W2, np.float64)
    W3f = np.asarray(W3, np.float64)

    def packw(WT):
        M = WT.shape[1]
        return np.ascontiguousarray(
            WT.reshape(3, 128, M).transpose(1, 0, 2).reshape(128, 3 * M)
        ).astype(BF16)

    common = {
        "xt": x,
        "w1a": packw(W1f[:, :H].T),
        "w1b": packw(W1f[:, H:].T),
        "w2": packw(W2f.T),
        "w3": packw(W3f.T),
        "b1": np.ascontiguousarray(
            np.asarray(b1, np.float32).reshape(3, 128).T),
        "b2": np.ascontiguousarray(
            (np.asarray(b2, np.float64) - W2f.sum(1)).astype(np.float32)
            .reshape(3, 128).T),
    }
    b3v = float(np.asarray(b3, np.float64)[0] - W3f.sum())
    common["b3"] = np.full((1, 1), b3v, np.float32)
    in_maps = [
        dict(common, sidx=wrap(src16[c]), didx=wrap(dst16[c]))
        for c in range(NCORES)
    ]
    return nbs, NT, b3v, in_maps, eid, E


def kernel(x1, x2, edge_index, W1, b1, W2, b2, W3, b3, backend="hw",
           run_kwargs=None):
    global LAST_RESULTS
    nbs, NT, b3v, in_maps, eid, E = _prep(
        x1, x2, edge_index, W1, b1, W2, b2, W3, b3)
    nc = _build_program(nbs, NT, b3v)

    if backend == "sim":
        from concourse.bass_interp import CoreSim
        per_core_out = []
        for c in range(NCORES):
            sim = CoreSim(nc)
            for k, v in in_maps[c].items():
                sim.tensor(k)[:] = v
            sim.simulate()
            per_core_out.append(np.array(sim.tensor("out")))
    else:
        from concourse.bass_utils import run_bass_kernel_spmd
        res = run_bass_kernel_spmd(
            nc, in_maps, core_ids=list(range(NCORES)), trace=TRACE,
            **(run_kwargs or {}))
        LAST_RESULTS = res
        per_core_out = [np.asarray(res.results[c]["out"]) for c in range(NCORES)]

    out = np.zeros(E, np.float32)
    for c in range(NCORES):
        o = per_core_out[c].reshape(-1)
        m = eid[c] >= 0
        out[eid[c][m]] = o[m]
    return out
